# revision 1
# baseline (speedup 1.0000x reference)
"""Trainium2 Bass kernel for nn_DCC_27006754357259 (dense transformer block).

Reference computation (per batch element b, S=2048 tokens, D=1024):
    xn   = ScaleNorm(x) * g
    uv   = silu(xn @ uv_w.T)            # [S, 2E+s], E=2048, s=128
    u, v, base = split(uv)
    q    = base*gamma0 + beta0 + pos;  k = base*gamma1 + beta1 + pos
    sc   = relu(q @ k.T / sqrt(s))^2
    out  = (u * (sc @ v)) @ o_w.T
    y    = x * res_scale + out

Sharding: pure data-parallel over batch B=8 -> 8 NeuronCores, one batch
element per core, no collectives.

Per-core dataflow (all matmuls bf16 with fp32 PSUM accumulation; measured
end-to-end error vs the fp32 reference ~3e-3 relative-to-absmax):
  P0: per token-tile: row sum-of-squares (ACT Square+accum), r = g/max(norm,eps),
      xn_bf16 = x*r, PE-transpose into feature-major xnT [D, S].
  P1: baseT = silu(uv_w_base @ xnT); qT/kT = baseT*gamma'+beta' + posT
      (1/sqrt(s) folded into gamma0/beta0/posq host-side).
  P2: v = silu(xnT.T @ uv_w_v) token-major, staged to DRAM (e-tile-major
      layout for fast column-block reads in P3).
  P3: per 512-token q-chunk: scoresT_j = relu(k_j @ qT)^2 (bf16);
      attnT_e = sum_j v_j,e @ scoresT_j; uT_e = silu(uv_w_u_e @ xnT);
      uaT_e = uT_e * attnT_e; out = uaT.T @ o_wT + x*res_scale.

Weight transposes (uv_w.T, o_w.T, pos_enc.T) and broadcasts are done on the
host inside kernel(); the device only ever does natural-layout DMAs.
"""
import math
from contextlib import ExitStack

import numpy as np
import ml_dtypes

import concourse.tile as tile
from concourse import bacc, mybir
from concourse.masks import make_identity

B, S, D = 8, 2048, 1024
E = 2 * D
SDIM = 128
EPS_LN = 1e-5
P = 128
KT = D // P          # 8 k-tiles over D
ST = S // P          # 16 token tiles
ET = E // P          # 16 e-tiles
NQC = 4              # 512-token q-chunks
QC = S // NQC
F32 = mybir.dt.float32
BF16 = mybir.dt.bfloat16


def build(nreps: int = 1):
    """Build + compile the per-core Bass program. nreps>1 wraps the body in a
    hardware For_i loop (used only for wall-clock calibration in test.py)."""
    nc = bacc.Bacc(None, target_bir_lowering=False)

    x_d = nc.dram_tensor("x", [S, D], F32, kind="ExternalInput")
    wu_d = nc.dram_tensor("wu", [D, E], BF16, kind="ExternalInput")     # uv_w[:E].T
    wv_d = nc.dram_tensor("wv", [D, E], BF16, kind="ExternalInput")     # uv_w[E:2E].T
    wb_d = nc.dram_tensor("wb", [D, SDIM], BF16, kind="ExternalInput")  # uv_w[2E:].T
    wo_d = nc.dram_tensor("wo", [E, D], BF16, kind="ExternalInput")     # o_w.T
    pq_d = nc.dram_tensor("pq", [SDIM, S], BF16, kind="ExternalInput")  # pos.T/sqrt(s)
    pk_d = nc.dram_tensor("pk", [SDIM, S], BF16, kind="ExternalInput")  # pos.T
    gb_d = nc.dram_tensor("gb", [P, 4], F32, kind="ExternalInput")      # g0',b0',g1,b1
    gl_d = nc.dram_tensor("gl", [P, 1], F32, kind="ExternalInput")      # ln_g bcast
    rs_d = nc.dram_tensor("rs", [P, D], F32, kind="ExternalInput")      # res_scale bcast
    y_d = nc.dram_tensor("y", [S, D], F32, kind="ExternalOutput")

    with TileProgram(nc, nreps) as tp:
        tp.emit(x_d, wu_d, wv_d, wb_d, wo_d, pq_d, pk_d, gb_d, gl_d, rs_d, y_d)
    nc.compile()
    return nc


class TileProgram:
    def __init__(self, nc, nreps):
        self.nc = nc
        self.nreps = nreps
        self.ctx = ExitStack()

    def __enter__(self):
        self.ctx.__enter__()
        self.tc = self.ctx.enter_context(tile.TileContext(self.nc))
        return self

    def __exit__(self, *exc):
        return self.ctx.__exit__(*exc)

    def emit(self, x_d, wu_d, wv_d, wb_d, wo_d, pq_d, pk_d, gb_d, gl_d, rs_d, y_d):
        nc, tc = self.nc, self.tc
        ctx = self.ctx

        perm = ctx.enter_context(tc.tile_pool(name="perm", bufs=1))
        # persistent tiles
        xnT = perm.tile([P, KT, S], BF16)            # 32 KB/part
        w_u = perm.tile([P, KT, E], BF16)            # 32
        w_o = perm.tile([P, ET, D], BF16)            # 32
        qT = perm.tile([P, S], BF16)                 # 4
        kT = perm.tile([P, S], BF16)                 # 4
        gb = perm.tile([P, 4], F32)
        gl = perm.tile([P, 1], F32)
        res = perm.tile([P, D], F32)                 # 4
        ident = perm.tile([P, P], BF16)

        nc.sync.dma_start(w_u[:], wu_d.ap().rearrange("(kt p) e -> p kt e", p=P))
        nc.sync.dma_start(w_o[:], wo_d.ap().rearrange("(et p) d -> p et d", p=P))
        nc.sync.dma_start(gb[:], gb_d.ap())
        nc.sync.dma_start(gl[:], gl_d.ap())
        nc.sync.dma_start(res[:], rs_d.ap())
        make_identity(nc, ident[:])

        dram = ctx.enter_context(tc.tile_pool(name="dram", bufs=1, space="DRAM"))
        v_dr = dram.tile([ET, S, P], BF16)           # v, e-tile-major

        if self.nreps > 1:
            loop = tc.For_i(0, self.nreps, 1)
            loop.__enter__()
        try:
            self.body(x_d, wv_d, wb_d, pq_d, pk_d, y_d,
                      xnT, w_u, w_o, qT, kT, gb, gl, res, ident, v_dr)
        finally:
            if self.nreps > 1:
                loop.__exit__(None, None, None)

    def body(self, x_d, wv_d, wb_d, pq_d, pk_d, y_d,
             xnT, w_u, w_o, qT, kT, gb, gl, res, ident, v_dr):
        nc, tc = self.nc, self.tc
        AF = mybir.ActivationFunctionType

        # ---------------- P0: norm + transpose -> xnT ----------------
        with (
            tc.tile_pool(name="p0", bufs=3) as p0,
            tc.tile_pool(name="p0s", bufs=4) as p0s,
            tc.tile_pool(name="ps0", bufs=4, space="PSUM") as ps0,
        ):
            trash = p0s.tile([P, D], F32, tag="trash")
            for st in range(ST):
                x_t = p0.tile([P, D], F32, tag="x")
                nc.sync.dma_start(x_t[:], x_d.ap()[st * P:(st + 1) * P, :])
                ss = p0s.tile([P, 1], F32, tag="ss")
                nc.scalar.activation(trash[:], x_t[:], AF.Square, accum_out=ss[:])
                nrm = p0s.tile([P, 1], F32, tag="nrm")
                nc.scalar.activation(nrm[:], ss[:], AF.Sqrt, scale=1.0 / D)
                nc.vector.tensor_scalar_max(nrm[:], nrm[:], EPS_LN)
                rin = p0s.tile([P, 1], F32, tag="rin")
                nc.vector.reciprocal(rin[:], nrm[:])
                r_t = p0s.tile([P, 1], F32, tag="r")
                nc.vector.tensor_mul(r_t[:], rin[:], gl[:])
                xn_b = p0.tile([P, D], BF16, tag="xn")
                nc.vector.tensor_scalar_mul(xn_b[:], x_t[:], r_t[:, 0:1])
                for kt in range(KT):
                    tp_ps = ps0.tile([P, P], BF16, tag="tp")
                    nc.tensor.transpose(tp_ps[:], xn_b[:, kt * P:(kt + 1) * P], ident[:])
                    nc.scalar.copy(xnT[:, kt, st * P:(st + 1) * P], tp_ps[:])

        # ---------------- P1: baseT -> qT, kT ----------------
        with (
            tc.tile_pool(name="p1", bufs=2) as p1,
            tc.tile_pool(name="p1c", bufs=1) as p1c,
            tc.tile_pool(name="ps1", bufs=2, space="PSUM") as ps1,
        ):
            w_b = p1c.tile([P, KT, SDIM], BF16)
            nc.sync.dma_start(w_b[:], wb_d.ap().rearrange("(kt p) s -> p kt s", p=P))
            posq = p1c.tile([P, S], BF16)
            posk = p1c.tile([P, S], BF16)
            nc.sync.dma_start(posq[:], pq_d.ap())
            nc.sync.dma_start(posk[:], pk_d.ap())
            for c in range(NQC):
                sl = slice(c * QC, (c + 1) * QC)
                ps_b = ps1.tile([P, QC], F32, tag="b")
                for kt in range(KT):
                    nc.tensor.matmul(ps_b[:], w_b[:, kt, :], xnT[:, kt, sl],
                                     start=(kt == 0), stop=(kt == KT - 1))
                baseT = p1.tile([P, QC], BF16, tag="base")
                nc.scalar.activation(baseT[:], ps_b[:], AF.Silu)
                nc.vector.tensor_scalar(qT[:, sl], baseT[:], gb[:, 0:1], gb[:, 1:2],
                                        op0=mybir.AluOpType.mult,
                                        op1=mybir.AluOpType.add)
                nc.vector.tensor_add(qT[:, sl], qT[:, sl], posq[:, sl])
                nc.vector.tensor_scalar(kT[:, sl], baseT[:], gb[:, 2:3], gb[:, 3:4],
                                        op0=mybir.AluOpType.mult,
                                        op1=mybir.AluOpType.add)
                nc.vector.tensor_add(kT[:, sl], kT[:, sl], posk[:, sl])

        # ---------------- P2: v -> DRAM (e-tile-major) ----------------
        with (
            tc.tile_pool(name="p2", bufs=2) as p2,
            tc.tile_pool(name="ps2", bufs=4, space="PSUM") as ps2,
        ):
            for ec in range(4):                      # 512-wide e-chunks
                wv_t = p2.tile([P, KT, 512], BF16, tag="wv")
                nc.sync.dma_start(
                    wv_t[:],
                    wv_d.ap()[:, ec * 512:(ec + 1) * 512]
                    .rearrange("(kt p) e -> p kt e", p=P))
                for st in range(ST):
                    ps_v = ps2.tile([P, 512], F32, tag="v")
                    for kt in range(KT):
                        nc.tensor.matmul(ps_v[:], xnT[:, kt, st * P:(st + 1) * P],
                                         wv_t[:, kt, :],
                                         start=(kt == 0), stop=(kt == KT - 1))
                    v_o = p2.tile([P, 4, P], BF16, tag="vo")
                    nc.scalar.activation(
                        v_o[:].rearrange("p c e -> p (c e)"), ps_v[:], AF.Silu)
                    # v_dr[ec*4+c, st*128+p, e] <- v_o[p, c, e]
                    nc.sync.dma_start(
                        v_dr[ec * 4:(ec + 1) * 4, st * P:(st + 1) * P, :]
                        .rearrange("c p e -> p c e"),
                        v_o[:])

        # ---------------- P3: attention + o-proj + residual ----------------
        with (
            tc.tile_pool(name="p3sc", bufs=2) as p3sc,
            tc.tile_pool(name="p3", bufs=3) as p3,
            tc.tile_pool(name="p3ua", bufs=2) as p3ua,
            tc.tile_pool(name="p3y", bufs=2) as p3y,
            tc.tile_pool(name="ps3", bufs=2, space="PSUM") as ps3,
        ):
            for qc in range(NQC):
                qsl = slice(qc * QC, (qc + 1) * QC)
                sc = p3sc.tile([P, ST, QC], BF16, tag="sc")
                for j in range(ST):
                    ps_s = ps3.tile([P, QC], F32, tag="s")
                    nc.tensor.matmul(ps_s[:], kT[:, j * P:(j + 1) * P], qT[:, qsl])
                    rl = p3.tile([P, QC], BF16, tag="rl")
                    nc.scalar.activation(rl[:], ps_s[:], AF.Relu)
                    nc.vector.tensor_mul(sc[:, j, :], rl[:], rl[:])
                ua = p3ua.tile([P, ET, QC], BF16, tag="ua")
                for et in range(ET):
                    vb = p3.tile([P, ST, P], BF16, tag="vb")
                    nc.sync.dma_start(
                        vb[:], v_dr[et].rearrange("(j p) e -> p j e", p=P))
                    ps_a = ps3.tile([P, QC], F32, tag="a")
                    for j in range(ST):
                        nc.tensor.matmul(ps_a[:], vb[:, j, :], sc[:, j, :],
                                         start=(j == 0), stop=(j == ST - 1))
                    ps_u = ps3.tile([P, QC], F32, tag="u")
                    for kt in range(KT):
                        nc.tensor.matmul(ps_u[:], w_u[:, kt, et * P:(et + 1) * P],
                                         xnT[:, kt, qsl],
                                         start=(kt == 0), stop=(kt == KT - 1))
                    ut = p3.tile([P, QC], BF16, tag="ut")
                    nc.scalar.activation(ut[:], ps_u[:], AF.Silu)
                    nc.vector.tensor_mul(ua[:, et, :], ut[:], ps_a[:])
                for qs in range(4):                  # 128-token sub-tiles
                    gq = qc * 4 + qs
                    x_r = p3y.tile([P, D], F32, tag="xr")
                    nc.sync.dma_start(x_r[:], x_d.ap()[gq * P:(gq + 1) * P, :])
                    y_t = p3y.tile([P, D], F32, tag="y")
                    nc.vector.tensor_mul(y_t[:], x_r[:], res[:])
                    for dc in range(2):              # 512-wide d chunks
                        ps_o = ps3.tile([P, 512], F32, tag="o")
                        for et in range(ET):
                            nc.tensor.matmul(
                                ps_o[:], ua[:, et, qs * P:(qs + 1) * P],
                                w_o[:, et, dc * 512:(dc + 1) * 512],
                                start=(et == 0), stop=(et == ET - 1))
                        dsl = slice(dc * 512, (dc + 1) * 512)
                        nc.vector.tensor_add(y_t[:, dsl], y_t[:, dsl], ps_o[:])
                    nc.sync.dma_start(y_d.ap()[gq * P:(gq + 1) * P, :], y_t[:])


_CACHED = {}


def _get_nc(nreps=1):
    if nreps not in _CACHED:
        _CACHED[nreps] = build(nreps)
    return _CACHED[nreps]


def prep_inputs(x, pos_enc, ln_g, uv_w, gamma, beta, o_w, res_scale):
    """Host-side: slice batch, transpose/cast weights, fold scales."""
    bf = ml_dtypes.bfloat16
    x = np.ascontiguousarray(np.asarray(x, np.float32))
    uv_w = np.asarray(uv_w, np.float32)
    o_w = np.asarray(o_w, np.float32)
    pos = np.asarray(pos_enc, np.float32)
    gamma = np.asarray(gamma, np.float32)
    beta = np.asarray(beta, np.float32)
    isq = 1.0 / math.sqrt(SDIM)

    wu = np.ascontiguousarray(uv_w[:E].T).astype(bf)            # [D, E]
    wv = np.ascontiguousarray(uv_w[E:2 * E].T).astype(bf)       # [D, E]
    wb = np.ascontiguousarray(uv_w[2 * E:].T).astype(bf)        # [D, SDIM]
    wo = np.ascontiguousarray(o_w.T).astype(bf)                 # [E, D]
    pq = np.ascontiguousarray(pos.T * isq).astype(bf)           # [SDIM, S]
    pk = np.ascontiguousarray(pos.T).astype(bf)                 # [SDIM, S]
    gb = np.stack([gamma[0] * isq, beta[0] * isq, gamma[1], beta[1]],
                  axis=1).astype(np.float32)                    # [SDIM, 4]
    gl = np.broadcast_to(np.asarray(ln_g, np.float32).reshape(1, 1),
                         (P, 1)).copy()
    rs = np.broadcast_to(np.asarray(res_scale, np.float32)[None, :],
                         (P, D)).copy()

    shared = {"wu": wu, "wv": wv, "wb": wb, "wo": wo, "pq": pq, "pk": pk,
              "gb": gb, "gl": gl, "rs": rs}
    return [dict(shared, x=x[b]) for b in range(B)]


def kernel(**inputs):
    from concourse.bass_utils import run_bass_kernel_spmd

    in_maps = prep_inputs(**inputs)
    nc = _get_nc(1)
    res = run_bass_kernel_spmd(nc, in_maps, core_ids=list(range(B)))
    out = np.stack([res.results[b]["y"] for b in range(B)], axis=0)
    return out.astype(np.float32)


# revision 2
# speedup vs baseline: 1.1055x; 1.1055x over previous
"""Trainium2 Bass kernel for nn_DCC_27006754357259 (dense transformer block).

Reference computation (per batch element b, S=2048 tokens, D=1024):
    xn   = ScaleNorm(x) * g
    uv   = silu(xn @ uv_w.T)            # [S, 2E+s], E=2048, s=128
    u, v, base = split(uv)
    q    = base*gamma0 + beta0 + pos;  k = base*gamma1 + beta1 + pos
    sc   = relu(q @ k.T / sqrt(s))^2
    out  = (u * (sc @ v)) @ o_w.T
    y    = x * res_scale + out

Sharding: pure data-parallel over batch B=8 -> 8 NeuronCores, one batch
element per core, no collectives.

All matmuls run in bf16 with fp32 PSUM accumulation (measured end-to-end
error vs the fp32 reference ~4e-3 relative-to-absmax). Weights arrive
host-transposed/cast; x arrives in both token-major (residual) and
feature-major xT (matmul) layouts.

Per-core dataflow, all activations kept feature-major [feature, token]:
  P0: per 512-token chunk: sq = xT*xT (bf16), ss_row[1,512] via ones-matmul
      over D, r_row = g/max(sqrt(ss/D),eps) on partition 0, broadcast to 128
      partitions via a rank-1 fp32 matmul into PSUM, xnT = xT * r_bc (bf16).
      (No PE transposes, no cross-partition reductions.)
  P1: baseT = silu(uv_w_base @ xnT); qT/kT = baseT*gamma'+beta' + posT
      (1/sqrt(s) folded into gamma0/beta0/posq host-side).
  P2: v = silu(xnT.T @ uv_w_v) token-major, staged to DRAM e-tile-major.
  P3: per 512-token q-chunk: scoresT_j = relu(k_j @ qT)^2 (bf16);
      attnT_e = sum_j v_j,e @ scoresT_j; uT_e = silu(uv_w_u_e @ xnT);
      uaT_e = uT_e * attnT_e; out = uaT.T @ o_wT + x*res_scale.
"""
import math
from contextlib import ExitStack

import numpy as np
import ml_dtypes

import concourse.tile as tile
from concourse import bacc, mybir
from concourse._compat import with_exitstack

B, S, D = 8, 2048, 1024
E = 2 * D
SDIM = 128
EPS_LN = 1e-5
P = 128
KT = D // P          # 8 k-tiles over D
ST = S // P          # 16 token tiles
ET = E // P          # 16 e-tiles
NQC = 4              # 512-token chunks
QC = S // NQC
F32 = mybir.dt.float32
BF16 = mybir.dt.bfloat16


def build(nreps: int = 1):
    """Build + compile the per-core Bass program. nreps>1 wraps the body in a
    hardware For_i loop (used only for wall-clock calibration in test.py)."""
    nc = bacc.Bacc(None, target_bir_lowering=False)

    x_d = nc.dram_tensor("x", [S, D], F32, kind="ExternalInput")
    xt_d = nc.dram_tensor("xt", [D, S], F32, kind="ExternalInput")      # x.T
    wu_d = nc.dram_tensor("wu", [D, E], BF16, kind="ExternalInput")     # uv_w[:E].T
    wv_d = nc.dram_tensor("wv", [D, E], BF16, kind="ExternalInput")     # uv_w[E:2E].T
    wb_d = nc.dram_tensor("wb", [D, SDIM], BF16, kind="ExternalInput")  # uv_w[2E:].T
    wo_d = nc.dram_tensor("wo", [E, D], BF16, kind="ExternalInput")     # o_w.T
    pq_d = nc.dram_tensor("pq", [SDIM, S], BF16, kind="ExternalInput")  # pos.T/sqrt(s)
    pk_d = nc.dram_tensor("pk", [SDIM, S], BF16, kind="ExternalInput")  # pos.T
    gb_d = nc.dram_tensor("gb", [P, 4], F32, kind="ExternalInput")      # g0',b0',g1,b1
    gl_d = nc.dram_tensor("gl", [P, 1], F32, kind="ExternalInput")      # ln_g bcast
    rs_d = nc.dram_tensor("rs", [P, D], F32, kind="ExternalInput")      # res_scale bcast
    y_d = nc.dram_tensor("y", [S, D], F32, kind="ExternalOutput")

    with ExitStack() as ctx:
        tc = ctx.enter_context(tile.TileContext(nc))
        perm = ctx.enter_context(tc.tile_pool(name="perm", bufs=1))
        xnT = perm.tile([P, KT, S], BF16)            # 32 KB/part
        w_u = perm.tile([P, KT, E], BF16)            # 32
        w_o = perm.tile([P, ET, D], BF16)            # 32
        qT = perm.tile([P, S], BF16)                 # 4
        kT = perm.tile([P, S], BF16)                 # 4
        gb = perm.tile([P, 4], F32)
        gl = perm.tile([P, 1], F32)
        res = perm.tile([P, D], F32)                 # 4
        ones_c = perm.tile([P, 1], BF16)             # ones column (ss matmul lhsT)
        ones_r = perm.tile([1, P], F32)              # ones row (broadcast lhsT)

        nc.sync.dma_start(w_u[:], wu_d.ap().rearrange("(kt p) e -> p kt e", p=P))
        nc.sync.dma_start(w_o[:], wo_d.ap().rearrange("(et p) d -> p et d", p=P))
        nc.sync.dma_start(gb[:], gb_d.ap())
        nc.sync.dma_start(gl[:], gl_d.ap())
        nc.sync.dma_start(res[:], rs_d.ap())
        nc.vector.memset(ones_c[:], 1.0)
        nc.vector.memset(ones_r[:], 1.0)

        dram = ctx.enter_context(tc.tile_pool(name="dram", bufs=1, space="DRAM"))
        v_dr = dram.tile([ET, S, P], BF16)           # v, e-tile-major

        loop = tc.For_i(0, nreps, 1) if nreps > 1 else None
        if loop is not None:
            loop.__enter__()
        try:
            _body(nc, tc, x_d, xt_d, wv_d, wb_d, pq_d, pk_d, y_d,
                  xnT, w_u, w_o, qT, kT, gb, gl, res, ones_c, ones_r, v_dr)
        finally:
            if loop is not None:
                loop.__exit__(None, None, None)
    nc.compile()
    return nc


def _body(nc, tc, x_d, xt_d, wv_d, wb_d, pq_d, pk_d, y_d,
          xnT, w_u, w_o, qT, kT, gb, gl, res, ones_c, ones_r, v_dr):
    AF = mybir.ActivationFunctionType

    # ---------------- P0: feature-major ScaleNorm -> xnT ----------------
    with (
        tc.tile_pool(name="p0", bufs=2) as p0,
        tc.tile_pool(name="p0s", bufs=4) as p0s,
        tc.tile_pool(name="ps0", bufs=2, space="PSUM") as ps0,
    ):
        for c in range(NQC):
            sl = slice(c * QC, (c + 1) * QC)
            xt_t = p0.tile([P, KT, QC], F32, tag="xt")
            nc.sync.dma_start(
                xt_t[:], xt_d.ap()[:, sl].rearrange("(kt p) s -> p kt s", p=P))
            sq = p0.tile([P, KT, QC], BF16, tag="sq")
            nc.vector.tensor_mul(sq[:], xt_t[:], xt_t[:])
            ps_ss = ps0.tile([1, QC], F32, tag="ss")
            for kt in range(KT):
                nc.tensor.matmul(ps_ss[:], ones_c[:], sq[:, kt, :],
                                 start=(kt == 0), stop=(kt == KT - 1))
            # partition-0 scalar chain: r = g / max(sqrt(ss/D), eps)
            nr = p0s.tile([1, QC], F32, tag="nr")
            nc.scalar.activation(nr[:], ps_ss[:], AF.Sqrt, scale=1.0 / D)
            nc.vector.tensor_scalar_max(nr[:], nr[:], EPS_LN)
            rr = p0s.tile([1, QC], F32, tag="rr")
            nc.vector.reciprocal(rr[:], nr[:])
            nc.vector.tensor_scalar_mul(rr[:], rr[:], gl[0:1, 0:1])
            ps_rb = ps0.tile([P, QC], F32, tag="rb")
            nc.tensor.matmul(ps_rb[:], ones_r[:], rr[:])
            for kt in range(KT):
                nc.vector.tensor_mul(xnT[:, kt, sl], xt_t[:, kt, :], ps_rb[:])

    # ---------------- P1: baseT -> qT, kT ----------------
    with (
        tc.tile_pool(name="p1", bufs=2) as p1,
        tc.tile_pool(name="p1c", bufs=1) as p1c,
        tc.tile_pool(name="ps1", bufs=2, space="PSUM") as ps1,
    ):
        w_b = p1c.tile([P, KT, SDIM], BF16)
        nc.sync.dma_start(w_b[:], wb_d.ap().rearrange("(kt p) s -> p kt s", p=P))
        posq = p1c.tile([P, S], BF16)
        posk = p1c.tile([P, S], BF16)
        nc.sync.dma_start(posq[:], pq_d.ap())
        nc.sync.dma_start(posk[:], pk_d.ap())
        for c in range(NQC):
            sl = slice(c * QC, (c + 1) * QC)
            ps_b = ps1.tile([P, QC], F32, tag="b")
            for kt in range(KT):
                nc.tensor.matmul(ps_b[:], w_b[:, kt, :], xnT[:, kt, sl],
                                 start=(kt == 0), stop=(kt == KT - 1))
            baseT = p1.tile([P, QC], BF16, tag="base")
            nc.scalar.activation(baseT[:], ps_b[:], AF.Silu)
            nc.vector.tensor_scalar(qT[:, sl], baseT[:], gb[:, 0:1], gb[:, 1:2],
                                    op0=mybir.AluOpType.mult,
                                    op1=mybir.AluOpType.add)
            nc.vector.tensor_add(qT[:, sl], qT[:, sl], posq[:, sl])
            nc.vector.tensor_scalar(kT[:, sl], baseT[:], gb[:, 2:3], gb[:, 3:4],
                                    op0=mybir.AluOpType.mult,
                                    op1=mybir.AluOpType.add)
            nc.vector.tensor_add(kT[:, sl], kT[:, sl], posk[:, sl])

    # ---------------- P2: v -> DRAM (e-tile-major) ----------------
    with (
        tc.tile_pool(name="p2", bufs=2) as p2,
        tc.tile_pool(name="ps2", bufs=4, space="PSUM") as ps2,
    ):
        for ec in range(4):                          # 512-wide e-chunks
            wv_t = p2.tile([P, KT, 512], BF16, tag="wv")
            nc.sync.dma_start(
                wv_t[:],
                wv_d.ap()[:, ec * 512:(ec + 1) * 512]
                .rearrange("(kt p) e -> p kt e", p=P))
            for st in range(ST):
                ps_v = ps2.tile([P, 512], F32, tag="v")
                for kt in range(KT):
                    nc.tensor.matmul(ps_v[:], xnT[:, kt, st * P:(st + 1) * P],
                                     wv_t[:, kt, :],
                                     start=(kt == 0), stop=(kt == KT - 1))
                v_o = p2.tile([P, 4, P], BF16, tag="vo")
                nc.scalar.activation(
                    v_o[:].rearrange("p c e -> p (c e)"), ps_v[:], AF.Silu)
                # v_dr[ec*4+c, st*128+p, e] <- v_o[p, c, e]
                nc.gpsimd.dma_start(
                    v_dr[ec * 4:(ec + 1) * 4, st * P:(st + 1) * P, :]
                    .rearrange("c p e -> p c e"),
                    v_o[:])

    # ---------------- P3: attention + o-proj + residual ----------------
    with (
        tc.tile_pool(name="p3sc", bufs=2) as p3sc,
        tc.tile_pool(name="p3", bufs=3) as p3,
        tc.tile_pool(name="p3ua", bufs=2) as p3ua,
        tc.tile_pool(name="p3y", bufs=2) as p3y,
        tc.tile_pool(name="ps3", bufs=2, space="PSUM") as ps3,
    ):
        for qc in range(NQC):
            qsl = slice(qc * QC, (qc + 1) * QC)
            sc = p3sc.tile([P, ST, QC], BF16, tag="sc")
            for j in range(ST):
                ps_s = ps3.tile([P, QC], F32, tag="s")
                nc.tensor.matmul(ps_s[:], kT[:, j * P:(j + 1) * P], qT[:, qsl])
                rl = p3.tile([P, QC], BF16, tag="rl")
                nc.scalar.activation(rl[:], ps_s[:], AF.Relu)
                nc.vector.tensor_mul(sc[:, j, :], rl[:], rl[:])
            ua = p3ua.tile([P, ET, QC], BF16, tag="ua")
            for et in range(ET):
                vb = p3.tile([P, ST, P], BF16, tag="vb")
                nc.scalar.dma_start(
                    vb[:], v_dr[et].rearrange("(j p) e -> p j e", p=P))
                ps_a = ps3.tile([P, QC], F32, tag="a")
                for j in range(ST):
                    nc.tensor.matmul(ps_a[:], vb[:, j, :], sc[:, j, :],
                                     start=(j == 0), stop=(j == ST - 1))
                ps_u = ps3.tile([P, QC], F32, tag="u")
                for kt in range(KT):
                    nc.tensor.matmul(ps_u[:], w_u[:, kt, et * P:(et + 1) * P],
                                     xnT[:, kt, qsl],
                                     start=(kt == 0), stop=(kt == KT - 1))
                ut = p3.tile([P, QC], BF16, tag="ut")
                nc.scalar.activation(ut[:], ps_u[:], AF.Silu)
                nc.vector.tensor_mul(ua[:, et, :], ut[:], ps_a[:])
            for qs in range(4):                      # 128-token sub-tiles
                gq = qc * 4 + qs
                x_r = p3y.tile([P, D], F32, tag="xr")
                nc.sync.dma_start(x_r[:], x_d.ap()[gq * P:(gq + 1) * P, :])
                y_t = p3y.tile([P, D], F32, tag="y")
                nc.vector.tensor_mul(y_t[:], x_r[:], res[:])
                for dc in range(2):                  # 512-wide d chunks
                    ps_o = ps3.tile([P, 512], F32, tag="o")
                    for et in range(ET):
                        nc.tensor.matmul(
                            ps_o[:], ua[:, et, qs * P:(qs + 1) * P],
                            w_o[:, et, dc * 512:(dc + 1) * 512],
                            start=(et == 0), stop=(et == ET - 1))
                    dsl = slice(dc * 512, (dc + 1) * 512)
                    nc.vector.tensor_add(y_t[:, dsl], y_t[:, dsl], ps_o[:])
                nc.sync.dma_start(y_d.ap()[gq * P:(gq + 1) * P, :], y_t[:])


_CACHED = {}


def _get_nc(nreps=1):
    if nreps not in _CACHED:
        _CACHED[nreps] = build(nreps)
    return _CACHED[nreps]


def prep_inputs(x, pos_enc, ln_g, uv_w, gamma, beta, o_w, res_scale):
    """Host-side: slice batch, transpose/cast weights, fold scales."""
    bf = ml_dtypes.bfloat16
    x = np.ascontiguousarray(np.asarray(x, np.float32))
    uv_w = np.asarray(uv_w, np.float32)
    o_w = np.asarray(o_w, np.float32)
    pos = np.asarray(pos_enc, np.float32)
    gamma = np.asarray(gamma, np.float32)
    beta = np.asarray(beta, np.float32)
    isq = 1.0 / math.sqrt(SDIM)

    wu = np.ascontiguousarray(uv_w[:E].T).astype(bf)            # [D, E]
    wv = np.ascontiguousarray(uv_w[E:2 * E].T).astype(bf)       # [D, E]
    wb = np.ascontiguousarray(uv_w[2 * E:].T).astype(bf)        # [D, SDIM]
    wo = np.ascontiguousarray(o_w.T).astype(bf)                 # [E, D]
    pq = np.ascontiguousarray(pos.T * isq).astype(bf)           # [SDIM, S]
    pk = np.ascontiguousarray(pos.T).astype(bf)                 # [SDIM, S]
    gb = np.stack([gamma[0] * isq, beta[0] * isq, gamma[1], beta[1]],
                  axis=1).astype(np.float32)                    # [SDIM, 4]
    gl = np.broadcast_to(np.asarray(ln_g, np.float32).reshape(1, 1),
                         (P, 1)).copy()
    rs = np.broadcast_to(np.asarray(res_scale, np.float32)[None, :],
                         (P, D)).copy()

    shared = {"wu": wu, "wv": wv, "wb": wb, "wo": wo, "pq": pq, "pk": pk,
              "gb": gb, "gl": gl, "rs": rs}
    return [dict(shared, x=x[b], xt=np.ascontiguousarray(x[b].T))
            for b in range(B)]


def kernel(**inputs):
    from concourse.bass_utils import run_bass_kernel_spmd

    in_maps = prep_inputs(**inputs)
    nc = _get_nc(1)
    res = run_bass_kernel_spmd(nc, in_maps, core_ids=list(range(B)))
    out = np.stack([res.results[b]["y"] for b in range(B)], axis=0)
    return out.astype(np.float32)
